# revision 32
# baseline (speedup 1.0000x reference)
"""Causal self-attention (GQA + rms_norm + RoPE) on 8 TRN2 NeuronCores.

Sharding: tensor-parallel over heads. Core c owns q-heads {2c, 2c+1} and
kv-head c//2 (GQA groups intact; each kv head is replicated on 2 cores).
Wo is sharded along its input (head) dim, so each core emits a partial
(T, C) bf16 output; the host sums the 8 partials in f32.

Per-core dataflow, software-pipelined over 512-row blocks bq:
  stage A (per 128-row t-tile): qkv = xT-tiles @ Wqkv (all bf16);
          v + the 3 q/k chunks evacuated from PSUM (ScalarE / DVE);
          RoPE on DVE in bf16 over the full 384-wide q|q|k strip using
          host-tripled cos/sin tables (sin first-half pre-negated so rope
          is mul/mul/add); rms stats via one bf16 square + one segmented
          free-axis reduce (rope preserves row norms); rsqrt via quake
          seed + 2 Newton steps on DVE; rstd applied to q only -- the
          k-side rstd is folded into the Exp scale in stage B; q,k
          transposed to [d, t] via XBAR DMA-transpose (no PE, no PSUM).
  stage B per head: S^T[s,tq] = kT_tile.T @ qT_block (bf16, diagonal
          tiles column-trimmed); P = exp(rstd_k[s]*scale*S^T) on ScalarE
          with a per-partition scale vector and no max-subtraction
          (rms_norm bounds |score*scale| <= sqrt(D) ~ 11.3); causal mask
          (+ garbage-column zeroing) on the 4 diagonal tiles via
          gpsimd.affine_select; PV (v_tile.T @ P^T) accumulates y^T in
          PSUM; the softmax row-sum is accumulated on DVE in bf16
          (elementwise adds across s-tiles) and reduced across partitions
          with a single ones-column matmul per (head, block).
  stage C: out[t,:] = sum_h yT_h.T @ Wo_h; PSUM evacuated to bf16 by
          gpsimd (Pool engine); DMA the bf16 partial to HBM.
"""

import math

import ml_dtypes
import numpy as np

import concourse.bass as bass
import concourse.mybir as mybir
import concourse.tile as tile
from concourse import bacc
from concourse.bass_utils import run_bass_kernel_spmd

F32 = mybir.dt.float32
BF16 = mybir.dt.bfloat16
MUL = mybir.AluOpType.mult
ADD = mybir.AluOpType.add

C = 2048          # model dim
H, KV, D = 16, 4, 128
REP = H // KV
N_CORES = 8
HPC = H // N_CORES          # q heads per core (2)
QKV_N = HPC * D + 2 * D     # qkv output columns per core (512)
NQK = (HPC + 1) * D         # q|q|k strip width (384)
EPS = 1e-6
SCALE = 1.0 / math.sqrt(D)
SKEW = 5


def build_nc(T: int) -> bass.Bass:
    assert T % 512 == 0
    n_tt = T // 128        # 128-row t-tiles
    n_blk = T // 512       # 512-wide tq blocks
    n_ct = C // 128        # contraction tiles for qkv

    nc = bacc.Bacc()
    # x is host-pre-tiled: xtiles[it*128+p, ct*128+t] = x[it*128+? ...] such
    # that each t-tile load is one contiguous 4KB segment per partition
    xT_d = nc.dram_tensor("xT", [T, C], BF16, kind="ExternalInput")
    wqkv_d = nc.dram_tensor("wqkv", [C, QKV_N], BF16, kind="ExternalInput")
    wo_d = nc.dram_tensor("wo", [HPC * D, C], BF16, kind="ExternalInput")
    cs_d = nc.dram_tensor("cs", [T, 6 * D], BF16, kind="ExternalInput")
    out_d = nc.dram_tensor("out", [T, C], BF16, kind="ExternalOutput")

    xT_r = xT_d[:].rearrange("(n p) c -> n p c", p=128)        # [n_tt, 128, C]
    wqkv_r = wqkv_d[:].rearrange("(ct p) n -> p ct n", p=128)  # [128, n_ct, 512]
    wo_r = wo_d[:].rearrange("(h p) n -> p h n", p=128)        # [128, HPC, C]
    cs_r = cs_d[:].rearrange("(n p) d -> n p d", p=128)        # [n_tt, 128, 768]
    out_r = out_d[:].rearrange("(n p) c -> n p c", p=128)      # [n_tt, 128, C]

    with tile.TileContext(nc) as tc:
        with (
            tc.tile_pool(name="singles", bufs=1) as singles,
            tc.tile_pool(name="xin", bufs=16) as xin,
            tc.tile_pool(name="csin", bufs=10) as csin,
            tc.tile_pool(name="qksb", bufs=3) as qksb,
            tc.tile_pool(name="abp", bufs=4) as abp,
            tc.tile_pool(name="rpp", bufs=12) as rpp,
            tc.tile_pool(name="small", bufs=4) as small,
            tc.tile_pool(name="ptp", bufs=8) as ptp,
            tc.tile_pool(name="accp", bufs=3) as accp,
            tc.tile_pool(name="ot", bufs=12) as otp,
            tc.tile_pool(name="ps", bufs=5, space="PSUM") as psp,
            tc.tile_pool(name="po", bufs=3, space="PSUM") as pop,
        ):
            # ---- constants / resident tensors ----
            ones_col = singles.tile([128, 1], BF16)
            nc.vector.memset(ones_col, 1.0)
            ones_row = singles.tile([1, 128], BF16)
            nc.vector.memset(ones_row, 1.0)
            # causal 0/1 masks for the 4 diagonal-tile offsets; mask d also
            # zeroes the [0, 128d) columns the trimmed score matmul skips
            masks = singles.tile([128, 4, 512], BF16)
            nc.vector.memset(masks, 1.0)
            for d in range(4):
                nc.gpsimd.affine_select(
                    out=masks[:, d, :], in_=masks[:, d, :],
                    compare_op=mybir.AluOpType.is_ge,
                    fill=0.0, base=-128 * d,
                    pattern=[[1, 512]], channel_multiplier=-1,
                )
            wqkv_s = singles.tile([128, n_ct, QKV_N], BF16)

            def load_wqkv(wc):
                sl = slice(wc * n_ct // 4, (wc + 1) * n_ct // 4)
                nc.sync.dma_start(out=wqkv_s[:, sl, :], in_=wqkv_r[:, sl, :])

            wo_s = singles.tile([128, HPC, C], BF16)

            qT = singles.tile([128, HPC, T], BF16)   # [d, h, t]
            kT = singles.tile([128, T], BF16)        # [d, s]
            v = singles.tile([128, n_tt, D], BF16)   # [s%128, s//128, d]
            yT = singles.tile([128, HPC, T], BF16)   # [d, h, t]
            rsk = singles.tile([128, n_tt], F32)     # SCALE * rstd_k per s-tile

            rp_store = {}
            pending_stores = []

            def stage_a_dma(it):
                """issue the input DMAs for t-tile it."""
                t0 = it * 128
                xt = xin.tile([128, n_ct, 128], BF16)
                nc.sync.dma_start(
                    out=xt,
                    in_=xT_r[it].rearrange("p (ct t) -> p ct t", t=128))
                cst = csin.tile([128, 6 * D], BF16)
                nc.sync.dma_start(out=cst, in_=cs_r[it])
                return xt, cst

            def stage_a_mm(it, pre):
                """qkv + rms + rope for t-tile it."""
                xt, cst = pre
                cos3 = cst[:, 0:NQK]
                snv3 = cst[:, NQK:2 * NQK]   # 3x [-sin[0:64] | sin[64:128]]

                ps = psp.tile([128, QKV_N], F32, tag="ps")
                for ct in range(n_ct):
                    nc.tensor.matmul(
                        ps, xt[:, ct, :], wqkv_s[:, ct, :],
                        start=(ct == 0), stop=(ct == n_ct - 1),
                    )
                # v: evacuate on ScalarE (cast to bf16)
                nc.scalar.copy(v[:, it, :], ps[:, NQK:QKV_N])
                # q|q|k strip: evacuate on DVE
                qk = qksb.tile([128, NQK], BF16, tag="qk")
                nc.vector.tensor_copy(qk, ps[:, 0:NQK])

                # rope in bf16 over the full strip
                a = abp.tile([128, NQK], BF16, tag="a")
                nc.vector.tensor_mul(a, qk, cos3)
                b = abp.tile([128, NQK], BF16, tag="b")
                swp = qk.rearrange(
                    "p (three two half) -> p three two half", three=3, two=2,
                )[:, :, ::-1, :]
                nc.vector.tensor_tensor(
                    out=b.rearrange(
                        "p (three two half) -> p three two half", three=3, two=2,
                    ),
                    in0=swp,
                    in1=snv3.rearrange(
                        "p (three two half) -> p three two half", three=3, two=2,
                    ),
                    op=MUL,
                )
                ab = abp.tile([128, NQK], BF16, tag="ab")
                nc.vector.tensor_add(ab, a, b)

                # rms stats: one bf16 square + segmented reduce
                sq = abp.tile([128, NQK], BF16, tag="sq")
                nc.vector.tensor_mul(sq, ab, ab)
                ssv = small.tile([128, HPC + 1], F32, tag="ssv")
                nc.vector.reduce_sum(
                    ssv[:].rearrange("p (three one) -> p three one", three=3),
                    sq.rearrange("p (three d) -> p three d", three=3),
                    axis=mybir.AxisListType.X,
                )
                nc.vector.tensor_scalar(
                    out=ssv, in0=ssv, scalar1=1.0 / D, scalar2=EPS,
                    op0=MUL, op1=ADD,
                )
                # rstd = rsqrt(ssv): quake seed + two Newton steps on DVE
                rstd = small.tile([128, HPC + 1], F32, tag="rstd")
                I32 = mybir.dt.int32
                nc.vector.tensor_scalar(
                    out=rstd.bitcast(I32), in0=ssv.bitcast(I32),
                    scalar1=1, scalar2=None,
                    op0=mybir.AluOpType.logical_shift_right,
                )
                nc.vector.tensor_scalar(
                    out=rstd.bitcast(I32), in0=rstd.bitcast(I32),
                    scalar1=0x5F3759DF, scalar2=-1,
                    op0=mybir.AluOpType.subtract, op1=MUL,
                )
                mh = small.tile([128, HPC + 1], F32, tag="mh")
                nc.vector.tensor_scalar(
                    out=mh, in0=ssv, scalar1=-0.5, scalar2=None, op0=MUL,
                )
                for _ in range(2):
                    u = small.tile([128, HPC + 1], F32, tag="u")
                    nc.vector.tensor_mul(u, rstd, rstd)
                    nc.vector.tensor_mul(u, u, mh)
                    nc.vector.tensor_scalar(
                        out=u, in0=u, scalar1=1.5, scalar2=None, op0=ADD,
                    )
                    nc.vector.tensor_mul(rstd, rstd, u)
                # k-side rstd folded into the stage-B exp scale
                nc.vector.tensor_scalar(
                    out=rsk[:, it:it + 1], in0=rstd[:, HPC:HPC + 1],
                    scalar1=SCALE, scalar2=None, op0=MUL,
                )
                # apply rstd to the q heads only
                rp = rpp.tile([128, HPC * D], BF16, tag="rp")
                for j in range(HPC):
                    nc.vector.tensor_scalar_mul(
                        rp[:, j * D:(j + 1) * D], ab[:, j * D:(j + 1) * D],
                        rstd[:, j:j + 1],
                    )
                rp_store[it] = (rp, ab)

            def stage_a_tp(it):
                """XBAR DMA-transpose q,k of t-tile it into qT/kT."""
                t0 = it * 128
                rp, ab = rp_store.pop(it)
                for j in range(HPC):
                    nc.sync.dma_start(
                        out=qT[:, j, t0:t0 + 128], in_=rp[:, j * D:(j + 1) * D],
                        transpose=True,
                    )
                nc.sync.dma_start(
                    out=kT[:, t0:t0 + 128], in_=ab[:, HPC * D:NQK],
                    transpose=True,
                )

            def stage_b(h, bq):
                """attention for (head h, tq block bq), skew pipelined."""
                q0 = bq * 512
                nst = 4 * bq + 4   # causal s-tiles
                pv = psp.tile([128, 512], F32, tag="ps")
                acc = accp.tile([128, 512], BF16, tag="acc")
                pts = {}

                def emit_score(st):
                    s0 = st * 128
                    off = max(0, 128 * (st - 4 * bq))  # diagonal column trim
                    sp = psp.tile([128, 512], F32, tag="ps")
                    nc.tensor.matmul(
                        sp[:, off:512], kT[:, s0:s0 + 128],
                        qT[:, h, q0 + off:q0 + 512],
                        start=True, stop=True,
                    )
                    pt = ptp.tile([128, 512], BF16, tag="pt")
                    # P = exp(SCALE * rstd_k[s] * S^T), valid columns only;
                    # pt[:, :off] is never read downstream
                    nc.scalar.activation(
                        pt[:, off:512], sp[:, off:512],
                        mybir.ActivationFunctionType.Exp,
                        scale=rsk[:, st:st + 1],
                    )
                    if st >= 4 * bq:  # diagonal tile: causal mask (DVE)
                        d = st - 4 * bq
                        nc.vector.tensor_mul(
                            pt[:, off:512], pt[:, off:512], masks[:, d, off:512])
                    pts[st] = (pt, off)

                def emit_consume(st):
                    pt, off = pts.pop(st)
                    nc.tensor.matmul(
                        pv[:, off:512], v[:, st, :], pt[:, off:512],
                        start=(st == 0), stop=(st == nst - 1),
                    )
                    if st == 0:
                        nc.vector.tensor_copy(acc, pt)
                    else:
                        # bf16 rowsum accumulate: per-partition chains round
                        # at 2^-9 but the final f32 partition-sum averages
                        # 128 independent chains -> ~3e-4 on the rowsum
                        with nc.allow_low_precision(reason="bf16 softmax rowsum"):
                            nc.vector.tensor_add(
                                acc[:, off:512], acc[:, off:512], pt[:, off:512])

                for st in range(nst):
                    emit_score(st)
                    if st >= SKEW:
                        emit_consume(st - SKEW)
                for st in range(max(0, nst - SKEW), nst):
                    emit_consume(st)
                return pv, acc

            def stage_b_fin(h, bq, pv, acc):
                """softmax normalization tail for (h, bq)."""
                q0 = bq * 512
                # partition-sum of the bf16 row-sum accumulator
                rsum = pop.tile([1, 512], F32, tag="po")
                nc.tensor.matmul(rsum, ones_col, acc, start=True, stop=True)
                rrs = small.tile([1, 512], BF16, tag="rrs")
                with nc.allow_low_precision(reason="bf16 1/rowsum for K=1 bcast"):
                    nc.vector.reciprocal(rrs, rsum)
                # broadcast 1/rowsum across partitions with a K=1 matmul
                bc = pop.tile([128, 512], F32, tag="po")
                nc.tensor.matmul(bc, ones_row, rrs, start=True, stop=True)
                nc.vector.tensor_mul(yT[:, h, q0:q0 + 512], pv, bc)

            def stage_c(it):
                """out = yT.T @ Wo (partial, bf16) for t-tile it.

                Fully decoupled background pipeline: own PSUM pool, Pool
                (gpsimd) evacuation, and the store DMA issued from Pool's
                software DGE so nothing here blocks the SP DMA queue or
                any attention-critical engine.
                """
                t0 = it * 128
                for nb in range(C // 512):
                    po = pop.tile([128, 512], F32, tag="po")
                    for hh in range(HPC):
                        nc.tensor.matmul(
                            po, yT[:, hh, t0:t0 + 128],
                            wo_s[:, hh, nb * 512:(nb + 1) * 512],
                            start=(hh == 0), stop=(hh == HPC - 1),
                        )
                    ot = otp.tile([128, 512], BF16)
                    nc.gpsimd.tensor_copy(ot, po)
                    # store DMA deferred one block so the in-order SP queue
                    # never blocks waiting on the Pool evacuation
                    pending_stores.append((it, nb, ot))

            def flush_stores():
                while pending_stores:
                    it, nb, ot = pending_stores.pop(0)
                    nc.sync.dma_start(
                        out=out_r[it][:, nb * 512:(nb + 1) * 512], in_=ot)

            # software pipeline over 512-row blocks; emission order shapes
            # each engine's static FIFO.
            pres = {}

            def emit_adma_group(b):
                if b < n_blk:
                    for it in range(4 * b, 4 * b + 4):
                        pres[it] = stage_a_dma(it)

            def emit_amm_group(b):
                if b < n_blk:
                    for it in range(4 * b, 4 * b + 4):
                        stage_a_mm(it, pres.pop(it))

            def emit_atp_group(b):
                if b < n_blk:
                    for it in range(4 * b, 4 * b + 4):
                        stage_a_tp(it)

            # prologue: interleave weight-chunk and x-tile loads so the
            # first qkv matmuls start as soon as chunk 0 + tile 0 land;
            # input DMA runs 3 blocks ahead of compute
            load_wqkv(0)
            pres[0] = stage_a_dma(0)
            for wc in range(1, 4):
                load_wqkv(wc)
            for it in range(1, 4):
                pres[it] = stage_a_dma(it)
            nc.sync.dma_start(out=wo_s, in_=wo_r)
            emit_adma_group(1)
            for it in range(4):
                stage_a_mm(it, pres.pop(it))
            emit_atp_group(0)
            emit_adma_group(2)
            emit_amm_group(1)
            emit_atp_group(1)
            emit_adma_group(3)
            emit_amm_group(2)
            emit_atp_group(2)
            fins = {(0, 0): stage_b(0, 0)}
            for bq in range(n_blk):
                flush_stores()
                emit_amm_group(bq + 3)
                emit_atp_group(bq + 3)
                emit_adma_group(bq + 4)
                fins[(1, bq)] = stage_b(1, bq)
                if bq + 1 < n_blk:
                    fins[(0, bq + 1)] = stage_b(0, bq + 1)
                # fins deferred past the next mains: PE absorbs the
                # recip/bcast latency with queued score work
                stage_b_fin(0, bq, *fins.pop((0, bq)))
                stage_b_fin(1, bq, *fins.pop((1, bq)))
                for it in range(4 * bq, 4 * bq + 4):
                    stage_c(it)
            flush_stores()

    nc.compile()
    return nc


_NC_CACHE: dict[int, bass.Bass] = {}


def _get_nc(T: int) -> bass.Bass:
    if T not in _NC_CACHE:
        _NC_CACHE[T] = build_nc(T)
    return _NC_CACHE[T]


def _rope_tables(T: int) -> np.ndarray:
    """[T, 6D] bf16 table: [cos x3 | snv x3], snv = [-sin[:, :D/2] | sin[:, D/2:]]."""
    inv_freq = 1.0 / (10000.0 ** (np.arange(0, D, 2, dtype=np.float64) / D))
    t = np.arange(T, dtype=np.float64)
    freqs = np.outer(t, inv_freq)
    emb = np.concatenate([freqs, freqs], axis=-1)
    cos = np.cos(emb)
    sin = np.sin(emb)
    snv = np.concatenate([-sin[:, :D // 2], sin[:, D // 2:]], axis=-1)
    return np.concatenate([cos] * 3 + [snv] * 3, axis=-1).astype(ml_dtypes.bfloat16)


def prepare_in_maps(x, Wq, Wk, Wv, Wo):
    B, T, _ = x.shape
    n_ct, n_tt = C // 128, T // 128
    xT = x.reshape(T, C).T.reshape(n_ct, 128, n_tt, 128)
    xT = np.ascontiguousarray(xT.transpose(2, 1, 0, 3).reshape(T, C)
                              ).astype(ml_dtypes.bfloat16)
    cs = _rope_tables(T)
    in_maps = []
    for c in range(N_CORES):
        g = c // 2
        h0 = c * HPC
        wqkv = np.ascontiguousarray(np.concatenate(
            [
                Wq[:, h0 * D:(h0 + HPC) * D],
                Wk[:, g * D:(g + 1) * D],
                Wv[:, g * D:(g + 1) * D],
            ],
            axis=1,
        )).astype(ml_dtypes.bfloat16)
        woc = np.ascontiguousarray(
            Wo[h0 * D:(h0 + HPC) * D, :]).astype(ml_dtypes.bfloat16)
        in_maps.append({"xT": xT, "wqkv": wqkv, "wo": woc, "cs": cs})
    return in_maps


def kernel(x, Wq, Wk, Wv, Wo):
    x = np.asarray(x, dtype=np.float32)
    Wq = np.asarray(Wq, dtype=np.float32)
    Wk = np.asarray(Wk, dtype=np.float32)
    Wv = np.asarray(Wv, dtype=np.float32)
    Wo = np.asarray(Wo, dtype=np.float32)
    B, T, _ = x.shape
    assert B == 1

    nc = _get_nc(T)
    in_maps = prepare_in_maps(x, Wq, Wk, Wv, Wo)
    res = run_bass_kernel_spmd(nc, in_maps, core_ids=list(range(N_CORES)))
    acc = np.zeros((T, C), dtype=np.float32)
    for r in res.results:
        acc += np.asarray(r["out"], dtype=np.float32)
    return acc.reshape(B, T, C)


# revision 33
# speedup vs baseline: 1.1101x; 1.1101x over previous
"""Causal self-attention (GQA + rms_norm + RoPE) on 8 TRN2 NeuronCores.

Sharding: tensor-parallel over heads. Core c owns q-heads {2c, 2c+1} and
kv-head c//2 (GQA groups intact; each kv head is replicated on 2 cores).
Wo is sharded along its input (head) dim, so each core emits a partial
(T, C) bf16 output; the host sums the 8 partials in f32.

Per-core dataflow, software-pipelined over 512-row blocks bq:
  stage A (per 128-row t-tile): qkv = xT-tiles @ Wqkv (all bf16);
          v + the 3 q/k chunks evacuated from PSUM (ScalarE / DVE);
          RoPE on DVE in bf16 over the full 384-wide q|q|k strip using
          host-tripled cos/sin tables (sin first-half pre-negated so rope
          is mul/mul/add); rms stats via one bf16 square + one segmented
          free-axis reduce (rope preserves row norms); rsqrt via quake
          seed + 2 Newton steps on DVE; rstd applied to q only -- the
          k-side rstd is folded into the Exp scale in stage B; q,k
          transposed to [d, t] via XBAR DMA-transpose (no PE, no PSUM).
  stage B per head: S^T[s,tq] = kT_tile.T @ qT_block (bf16, diagonal
          tiles column-trimmed); P = exp(rstd_k[s]*scale*S^T) on ScalarE
          with a per-partition scale vector and no max-subtraction
          (rms_norm bounds |score*scale| <= sqrt(D) ~ 11.3); causal mask
          (+ garbage-column zeroing) on the 4 diagonal tiles via
          gpsimd.affine_select; PV (v_tile.T @ P^T) accumulates y^T in
          PSUM; the softmax row-sum is accumulated on DVE in bf16
          (elementwise adds across s-tiles) and reduced across partitions
          with a single ones-column matmul per (head, block).
  stage C: out[t,:] = sum_h yT_h.T @ Wo_h; PSUM evacuated to bf16 by
          gpsimd (Pool engine); DMA the bf16 partial to HBM.
"""

import math

import ml_dtypes
import numpy as np

import concourse.bass as bass
import concourse.mybir as mybir
import concourse.tile as tile
from concourse import bacc
from concourse.bass_utils import run_bass_kernel_spmd

F32 = mybir.dt.float32
BF16 = mybir.dt.bfloat16
MUL = mybir.AluOpType.mult
ADD = mybir.AluOpType.add

C = 2048          # model dim
H, KV, D = 16, 4, 128
REP = H // KV
N_CORES = 8
HPC = H // N_CORES          # q heads per core (2)
QKV_N = HPC * D + 2 * D     # qkv output columns per core (512)
NQK = (HPC + 1) * D         # q|q|k strip width (384)
EPS = 1e-6
SCALE = 1.0 / math.sqrt(D)
SKEW = 5


def build_nc(T: int) -> bass.Bass:
    assert T % 512 == 0
    n_tt = T // 128        # 128-row t-tiles
    n_blk = T // 512       # 512-wide tq blocks
    n_ct = C // 128        # contraction tiles for qkv

    nc = bacc.Bacc()
    # x is host-pre-tiled: xtiles[it*128+p, ct*128+t] = x[it*128+? ...] such
    # that each t-tile load is one contiguous 4KB segment per partition
    xT_d = nc.dram_tensor("xT", [T, C], BF16, kind="ExternalInput")
    wqkv_d = nc.dram_tensor("wqkv", [C, QKV_N], BF16, kind="ExternalInput")
    wo_d = nc.dram_tensor("wo", [HPC * D, C], BF16, kind="ExternalInput")
    cs_d = nc.dram_tensor("cs", [T, 6 * D], BF16, kind="ExternalInput")
    out_d = nc.dram_tensor("out", [T, C], BF16, kind="ExternalOutput")

    xT_r = xT_d[:].rearrange("(n p) c -> n p c", p=128)        # [n_tt, 128, C]
    wqkv_r = wqkv_d[:].rearrange("(ct p) n -> p ct n", p=128)  # [128, n_ct, 512]
    wo_r = wo_d[:].rearrange("(h p) n -> p h n", p=128)        # [128, HPC, C]
    cs_r = cs_d[:].rearrange("(n p) d -> n p d", p=128)        # [n_tt, 128, 768]
    out_r = out_d[:].rearrange("(n p) c -> n p c", p=128)      # [n_tt, 128, C]

    with tile.TileContext(nc) as tc:
        with (
            tc.tile_pool(name="singles", bufs=1) as singles,
            tc.tile_pool(name="xin", bufs=12) as xin,
            tc.tile_pool(name="csin", bufs=12) as csin,
            tc.tile_pool(name="qksb", bufs=3) as qksb,
            tc.tile_pool(name="abp", bufs=4) as abp,
            tc.tile_pool(name="rpp", bufs=8) as rpp,
            tc.tile_pool(name="small", bufs=4) as small,
            tc.tile_pool(name="ptp", bufs=10) as ptp,
            tc.tile_pool(name="accp", bufs=3) as accp,
            tc.tile_pool(name="ot", bufs=18) as otp,
            tc.tile_pool(name="ps", bufs=5, space="PSUM") as psp,
            tc.tile_pool(name="po", bufs=3, space="PSUM") as pop,
        ):
            # ---- constants / resident tensors ----
            ones_col = singles.tile([128, 1], BF16)
            nc.vector.memset(ones_col, 1.0)
            ones_row = singles.tile([1, 128], BF16)
            nc.vector.memset(ones_row, 1.0)
            # causal 0/1 masks for the 4 diagonal-tile offsets; mask d also
            # zeroes the [0, 128d) columns the trimmed score matmul skips
            masks = singles.tile([128, 4, 512], BF16)
            nc.vector.memset(masks, 1.0)
            for d in range(4):
                nc.gpsimd.affine_select(
                    out=masks[:, d, :], in_=masks[:, d, :],
                    compare_op=mybir.AluOpType.is_ge,
                    fill=0.0, base=-128 * d,
                    pattern=[[1, 512]], channel_multiplier=-1,
                )
            wqkv_s = singles.tile([128, n_ct, QKV_N], BF16)

            def load_wqkv(wc):
                sl = slice(wc * n_ct // 4, (wc + 1) * n_ct // 4)
                nc.sync.dma_start(out=wqkv_s[:, sl, :], in_=wqkv_r[:, sl, :])

            wo_s = singles.tile([128, HPC, C], BF16)

            qT = singles.tile([128, HPC, T], BF16)   # [d, h, t]
            kT = singles.tile([128, T], BF16)        # [d, s]
            v = singles.tile([128, n_tt, D], BF16)   # [s%128, s//128, d]
            yT = singles.tile([128, HPC, T], BF16)   # [d, h, t]
            rsk = singles.tile([128, n_tt], F32)     # SCALE * rstd_k per s-tile

            rp_store = {}
            pending_stores = []

            def stage_a_dma(it):
                """issue the input DMAs for t-tile it."""
                t0 = it * 128
                xt = xin.tile([128, n_ct, 128], BF16)
                nc.sync.dma_start(
                    out=xt,
                    in_=xT_r[it].rearrange("p (ct t) -> p ct t", t=128))
                cst = csin.tile([128, 6 * D], BF16)
                nc.sync.dma_start(out=cst, in_=cs_r[it])
                return xt, cst

            def stage_a_mm(it, pre):
                """qkv + rms + rope for t-tile it."""
                xt, cst = pre
                cos3 = cst[:, 0:NQK]
                snv3 = cst[:, NQK:2 * NQK]   # 3x [-sin[0:64] | sin[64:128]]

                ps = psp.tile([128, QKV_N], F32, tag="ps")
                for ct in range(n_ct):
                    nc.tensor.matmul(
                        ps, xt[:, ct, :], wqkv_s[:, ct, :],
                        start=(ct == 0), stop=(ct == n_ct - 1),
                    )
                # v: evacuate on ScalarE (cast to bf16)
                nc.scalar.copy(v[:, it, :], ps[:, NQK:QKV_N])
                # q|q|k strip: evacuate on DVE
                qk = qksb.tile([128, NQK], BF16, tag="qk")
                nc.vector.tensor_copy(qk, ps[:, 0:NQK])

                # rope in bf16 over the full strip
                a = abp.tile([128, NQK], BF16, tag="a")
                nc.vector.tensor_mul(a, qk, cos3)
                b = abp.tile([128, NQK], BF16, tag="b")
                swp = qk.rearrange(
                    "p (three two half) -> p three two half", three=3, two=2,
                )[:, :, ::-1, :]
                nc.vector.tensor_tensor(
                    out=b.rearrange(
                        "p (three two half) -> p three two half", three=3, two=2,
                    ),
                    in0=swp,
                    in1=snv3.rearrange(
                        "p (three two half) -> p three two half", three=3, two=2,
                    ),
                    op=MUL,
                )
                ab = abp.tile([128, NQK], BF16, tag="ab")
                nc.vector.tensor_add(ab, a, b)

                # rms stats: one bf16 square + segmented reduce
                sq = abp.tile([128, NQK], BF16, tag="sq")
                nc.vector.tensor_mul(sq, ab, ab)
                ssv = small.tile([128, HPC + 1], F32, tag="ssv")
                nc.vector.reduce_sum(
                    ssv[:].rearrange("p (three one) -> p three one", three=3),
                    sq.rearrange("p (three d) -> p three d", three=3),
                    axis=mybir.AxisListType.X,
                )
                nc.vector.tensor_scalar(
                    out=ssv, in0=ssv, scalar1=1.0 / D, scalar2=EPS,
                    op0=MUL, op1=ADD,
                )
                # rstd = rsqrt(ssv): quake seed + two Newton steps on DVE
                rstd = small.tile([128, HPC + 1], F32, tag="rstd")
                I32 = mybir.dt.int32
                nc.vector.tensor_scalar(
                    out=rstd.bitcast(I32), in0=ssv.bitcast(I32),
                    scalar1=1, scalar2=None,
                    op0=mybir.AluOpType.logical_shift_right,
                )
                nc.vector.tensor_scalar(
                    out=rstd.bitcast(I32), in0=rstd.bitcast(I32),
                    scalar1=0x5F3759DF, scalar2=-1,
                    op0=mybir.AluOpType.subtract, op1=MUL,
                )
                mh = small.tile([128, HPC + 1], F32, tag="mh")
                nc.vector.tensor_scalar(
                    out=mh, in0=ssv, scalar1=-0.5, scalar2=None, op0=MUL,
                )
                for _ in range(2):
                    u = small.tile([128, HPC + 1], F32, tag="u")
                    nc.vector.tensor_mul(u, rstd, rstd)
                    nc.vector.tensor_mul(u, u, mh)
                    nc.vector.tensor_scalar(
                        out=u, in0=u, scalar1=1.5, scalar2=None, op0=ADD,
                    )
                    nc.vector.tensor_mul(rstd, rstd, u)
                # k-side rstd folded into the stage-B exp scale
                nc.vector.tensor_scalar(
                    out=rsk[:, it:it + 1], in0=rstd[:, HPC:HPC + 1],
                    scalar1=SCALE, scalar2=None, op0=MUL,
                )
                # apply rstd to the q heads only
                rp = rpp.tile([128, HPC * D], BF16, tag="rp")
                for j in range(HPC):
                    nc.vector.tensor_scalar_mul(
                        rp[:, j * D:(j + 1) * D], ab[:, j * D:(j + 1) * D],
                        rstd[:, j:j + 1],
                    )
                rp_store[it] = (rp, ab)

            def stage_a_tp(it):
                """XBAR DMA-transpose q,k of t-tile it into qT/kT."""
                t0 = it * 128
                rp, ab = rp_store.pop(it)
                for j in range(HPC):
                    nc.sync.dma_start(
                        out=qT[:, j, t0:t0 + 128], in_=rp[:, j * D:(j + 1) * D],
                        transpose=True,
                    )
                nc.sync.dma_start(
                    out=kT[:, t0:t0 + 128], in_=ab[:, HPC * D:NQK],
                    transpose=True,
                )

            def stage_b(h, bq):
                """attention for (head h, tq block bq), skew pipelined."""
                q0 = bq * 512
                nst = 4 * bq + 4   # causal s-tiles
                pv = psp.tile([128, 512], F32, tag="ps")
                acc = accp.tile([128, 512], BF16, tag="acc")
                pts = {}

                def emit_score(st):
                    s0 = st * 128
                    off = max(0, 128 * (st - 4 * bq))  # diagonal column trim
                    sp = psp.tile([128, 512], F32, tag="ps")
                    nc.tensor.matmul(
                        sp[:, off:512], kT[:, s0:s0 + 128],
                        qT[:, h, q0 + off:q0 + 512],
                        start=True, stop=True,
                    )
                    pt = ptp.tile([128, 512], BF16, tag="pt")
                    # P = exp(SCALE * rstd_k[s] * S^T), valid columns only;
                    # pt[:, :off] is never read downstream
                    nc.scalar.activation(
                        pt[:, off:512], sp[:, off:512],
                        mybir.ActivationFunctionType.Exp,
                        scale=rsk[:, st:st + 1],
                    )
                    if st >= 4 * bq:  # diagonal tile: causal mask (DVE)
                        d = st - 4 * bq
                        nc.vector.tensor_mul(
                            pt[:, off:512], pt[:, off:512], masks[:, d, off:512])
                    pts[st] = (pt, off)

                def emit_consume(st):
                    pt, off = pts.pop(st)
                    nc.tensor.matmul(
                        pv[:, off:512], v[:, st, :], pt[:, off:512],
                        start=(st == 0), stop=(st == nst - 1),
                    )
                    if st == 0:
                        nc.vector.tensor_copy(acc, pt)
                    else:
                        # bf16 rowsum accumulate: per-partition chains round
                        # at 2^-9 but the final f32 partition-sum averages
                        # 128 independent chains -> ~3e-4 on the rowsum
                        with nc.allow_low_precision(reason="bf16 softmax rowsum"):
                            nc.vector.tensor_add(
                                acc[:, off:512], acc[:, off:512], pt[:, off:512])

                for st in range(nst):
                    emit_score(st)
                    if st >= SKEW:
                        emit_consume(st - SKEW)
                for st in range(max(0, nst - SKEW), nst):
                    emit_consume(st)
                return pv, acc

            def stage_b_fin(h, bq, pv, acc):
                """softmax normalization tail for (h, bq)."""
                q0 = bq * 512
                # partition-sum of the bf16 row-sum accumulator
                rsum = pop.tile([1, 512], F32, tag="po")
                nc.tensor.matmul(rsum, ones_col, acc, start=True, stop=True)
                rrs = small.tile([1, 512], BF16, tag="rrs")
                with nc.allow_low_precision(reason="bf16 1/rowsum for K=1 bcast"):
                    nc.vector.reciprocal(rrs, rsum)
                # broadcast 1/rowsum across partitions with a K=1 matmul
                bc = pop.tile([128, 512], F32, tag="po")
                nc.tensor.matmul(bc, ones_row, rrs, start=True, stop=True)
                nc.vector.tensor_mul(yT[:, h, q0:q0 + 512], pv, bc)

            def stage_c(it):
                """out = yT.T @ Wo (partial, bf16) for t-tile it.

                Fully decoupled background pipeline: own PSUM pool, Pool
                (gpsimd) evacuation, and the store DMA issued from Pool's
                software DGE so nothing here blocks the SP DMA queue or
                any attention-critical engine.
                """
                t0 = it * 128
                for nb in range(C // 512):
                    po = pop.tile([128, 512], F32, tag="po")
                    for hh in range(HPC):
                        nc.tensor.matmul(
                            po, yT[:, hh, t0:t0 + 128],
                            wo_s[:, hh, nb * 512:(nb + 1) * 512],
                            start=(hh == 0), stop=(hh == HPC - 1),
                        )
                    ot = otp.tile([128, 512], BF16)
                    nc.gpsimd.tensor_copy(ot, po)
                    # store DMA deferred one block so the in-order SP queue
                    # never blocks waiting on the Pool evacuation
                    pending_stores.append((it, nb, ot))

            def flush_stores():
                while pending_stores:
                    it, nb, ot = pending_stores.pop(0)
                    nc.sync.dma_start(
                        out=out_r[it][:, nb * 512:(nb + 1) * 512], in_=ot)

            # software pipeline over 512-row blocks; emission order shapes
            # each engine's static FIFO.
            pres = {}

            def emit_adma_group(b):
                if b < n_blk:
                    for it in range(4 * b, 4 * b + 4):
                        pres[it] = stage_a_dma(it)

            def emit_amm_group(b):
                if b < n_blk:
                    for it in range(4 * b, 4 * b + 4):
                        stage_a_mm(it, pres.pop(it))

            def emit_atp_group(b):
                if b < n_blk:
                    for it in range(4 * b, 4 * b + 4):
                        stage_a_tp(it)

            # prologue: interleave weight-chunk and x-tile loads so the
            # first qkv matmuls start as soon as chunk 0 + tile 0 land;
            # input DMA runs 3 blocks ahead of compute
            load_wqkv(0)
            pres[0] = stage_a_dma(0)
            for wc in range(1, 4):
                load_wqkv(wc)
            for it in range(1, 4):
                pres[it] = stage_a_dma(it)
            nc.sync.dma_start(out=wo_s, in_=wo_r)
            emit_adma_group(1)
            for it in range(4):
                stage_a_mm(it, pres.pop(it))
            emit_atp_group(0)
            emit_adma_group(2)
            emit_amm_group(1)
            emit_atp_group(1)
            fins = {(0, 0): stage_b(0, 0)}
            for bq in range(n_blk):
                flush_stores()
                emit_amm_group(bq + 2)
                emit_atp_group(bq + 2)
                emit_adma_group(bq + 3)
                fins[(1, bq)] = stage_b(1, bq)
                if bq + 1 < n_blk:
                    fins[(0, bq + 1)] = stage_b(0, bq + 1)
                # fins deferred past the next mains: PE absorbs the
                # recip/bcast latency with queued score work
                stage_b_fin(0, bq, *fins.pop((0, bq)))
                stage_b_fin(1, bq, *fins.pop((1, bq)))
                for it in range(4 * bq, 4 * bq + 4):
                    stage_c(it)
            flush_stores()

    nc.compile()
    return nc


_NC_CACHE: dict[int, bass.Bass] = {}


def _get_nc(T: int) -> bass.Bass:
    if T not in _NC_CACHE:
        _NC_CACHE[T] = build_nc(T)
    return _NC_CACHE[T]


def _rope_tables(T: int) -> np.ndarray:
    """[T, 6D] bf16 table: [cos x3 | snv x3], snv = [-sin[:, :D/2] | sin[:, D/2:]]."""
    inv_freq = 1.0 / (10000.0 ** (np.arange(0, D, 2, dtype=np.float64) / D))
    t = np.arange(T, dtype=np.float64)
    freqs = np.outer(t, inv_freq)
    emb = np.concatenate([freqs, freqs], axis=-1)
    cos = np.cos(emb)
    sin = np.sin(emb)
    snv = np.concatenate([-sin[:, :D // 2], sin[:, D // 2:]], axis=-1)
    return np.concatenate([cos] * 3 + [snv] * 3, axis=-1).astype(ml_dtypes.bfloat16)


def prepare_in_maps(x, Wq, Wk, Wv, Wo):
    B, T, _ = x.shape
    n_ct, n_tt = C // 128, T // 128
    xT = x.reshape(T, C).T.reshape(n_ct, 128, n_tt, 128)
    xT = np.ascontiguousarray(xT.transpose(2, 1, 0, 3).reshape(T, C)
                              ).astype(ml_dtypes.bfloat16)
    cs = _rope_tables(T)
    in_maps = []
    for c in range(N_CORES):
        g = c // 2
        h0 = c * HPC
        wqkv = np.ascontiguousarray(np.concatenate(
            [
                Wq[:, h0 * D:(h0 + HPC) * D],
                Wk[:, g * D:(g + 1) * D],
                Wv[:, g * D:(g + 1) * D],
            ],
            axis=1,
        )).astype(ml_dtypes.bfloat16)
        woc = np.ascontiguousarray(
            Wo[h0 * D:(h0 + HPC) * D, :]).astype(ml_dtypes.bfloat16)
        in_maps.append({"xT": xT, "wqkv": wqkv, "wo": woc, "cs": cs})
    return in_maps


def kernel(x, Wq, Wk, Wv, Wo):
    x = np.asarray(x, dtype=np.float32)
    Wq = np.asarray(Wq, dtype=np.float32)
    Wk = np.asarray(Wk, dtype=np.float32)
    Wv = np.asarray(Wv, dtype=np.float32)
    Wo = np.asarray(Wo, dtype=np.float32)
    B, T, _ = x.shape
    assert B == 1

    nc = _get_nc(T)
    in_maps = prepare_in_maps(x, Wq, Wk, Wv, Wo)
    res = run_bass_kernel_spmd(nc, in_maps, core_ids=list(range(N_CORES)))
    acc = np.zeros((T, C), dtype=np.float32)
    for r in res.results:
        acc += np.asarray(r["out"], dtype=np.float32)
    return acc.reshape(B, T, C)
